# revision 32
# baseline (speedup 1.0000x reference)
"""Tensor-parallel MHSA (RoPE + causal attention) for 8 TRN2 NeuronCores.

Sharding: 8-way tensor-parallel over heads (16 heads -> 2 per core).
Each core computes q/k/v projections for its 2 heads (column-parallel),
RoPE, causal attention, and a row-parallel slice of the output projection,
producing a full-shape partial y^T (bf16); the host sums the 8 partials.

Layout/schedule notes (~334 us on the TimelineSim cost model, vs 596 us
for the f32r baseline; PE-work floor is ~320 us):
- All matmul operands bf16 (full PE rate at any free-dim size); PSUM
  accumulation stays f32.  Scores are computed transposed (S^T[m, l]) so
  softmax sums are ones-vector matmuls and A@V needs no transposes.
- All PSUM pools are global; banks time-share across phases by tag
  (PSUM slots are bank-quantized -- every tile costs a full 2KB bank):
  A: q/k accumulators <-> attention st tiles (4 banks)
  B: v accumulators  <-> out-proj yp tiles  (2 banks)
  av (1 bank), rs (1 bank).
- QKV projection in 512-token chunks, three passes (q, k, v); startup
  DMAs interleave wq/x quarters so PE starts ~2 us in.
- Attention runs heads sequentially with a depth-3 software pipeline
  (st emitted 3 blocks ahead of av/rs); the first 3 st/exp blocks are
  prefilled into the tail of the QKV v-pass so attention starts hot.
- The causal mask is added on PE (identity-matmul accumulate into the
  diagonal st block), keeping the st->exp->av chain off DVE.
- Out-projection tiles go through a pending queue, drained one eb-PAIR
  (one DMA descriptor) at a time into attention iterations / head gaps /
  the next batch's QKV passes, so copy+DMA never limits PE; the final
  burst alternates A/B banks and Act/DVE copiers.
- The softmax denominator accumulates on PE as a broadcast row-sum
  (lhsT=ones[128,128]), so normalization is reciprocal+mul on DVE only.
- y streams out as bf16 partials; host sums in f32 and adds bo.
"""
import sys
sys.path.insert(0, "/opt/trn_rl_repo")
import numpy as np

B, L, E = 2, 2048, 2048
HEADS = 16
HD = 128
BASE = 10000.0
NCORES = 8
HPC = HEADS // NCORES      # heads per core = 2
COLS = HPC * HD            # 256 columns of Wq/Wk/Wv per core
KT = E // 128              # 16 k-tiles
LC = L // 512              # 4 l-chunks (qkv + attention + out-proj)
NEG = -1.0e9


def _build_program():
    import concourse.bass as bass
    import concourse.mybir as mybir
    import concourse.tile as tile
    from concourse import bacc
    from concourse.alu_op_type import AluOpType

    F32 = mybir.dt.float32
    BF16 = mybir.dt.bfloat16
    Exp = mybir.ActivationFunctionType.Exp

    nc = bacc.Bacc()
    xT_d = nc.declare_dram_parameter("xT", [B, E, L], BF16, isOutput=False)
    wq_d = nc.declare_dram_parameter("wq", [E, COLS], BF16, isOutput=False)
    wk_d = nc.declare_dram_parameter("wk", [E, COLS], BF16, isOutput=False)
    wv_d = nc.declare_dram_parameter("wv", [E, COLS], BF16, isOutput=False)
    wo_d = nc.declare_dram_parameter("wo", [COLS, E], BF16, isOutput=False)
    bq_d = nc.declare_dram_parameter("bq", [1, COLS], BF16, isOutput=False)
    bk_d = nc.declare_dram_parameter("bk", [1, COLS], BF16, isOutput=False)
    bv_d = nc.declare_dram_parameter("bv", [1, COLS], BF16, isOutput=False)
    cos_d = nc.declare_dram_parameter("cosf", [64, L], F32, isOutput=False)
    sin_d = nc.declare_dram_parameter("sinf", [64, L], F32, isOutput=False)
    mask_d = nc.declare_dram_parameter("mask", [128, 128], BF16, isOutput=False)
    id_d = nc.declare_dram_parameter("idm", [128, 128], BF16, isOutput=False)
    ones_d = nc.declare_dram_parameter("ones", [128, 512], BF16, isOutput=False)
    y_d = nc.declare_dram_parameter("yT", [B, E, L], BF16, isOutput=True)

    with nc.allow_low_precision(reason="bf16 matmuls"), \
         tile.TileContext(nc) as tc:
        with (
            tc.tile_pool(name="fixed", bufs=1) as fixed,
            tc.tile_pool(name="qkv", bufs=1) as qkvp,
            tc.tile_pool(name="xs", bufs=2) as xs,
            tc.tile_pool(name="pt", bufs=6) as ptp,
            tc.tile_pool(name="ys", bufs=8) as ysp,
            tc.tile_pool(name="small", bufs=4) as smallp,
            tc.tile_pool(name="rope", bufs=2) as ropep,
            tc.tile_pool(name="pA", bufs=4, space="PSUM") as pA,
            tc.tile_pool(name="pB", bufs=2, space="PSUM") as pB,
            tc.tile_pool(name="pav", bufs=1, space="PSUM") as pav,
            tc.tile_pool(name="prs", bufs=1, space="PSUM") as prs,
        ):
            # ---- DMA order: wq halves, x(b0,c0) halves, wk, cos, sin,
            #      ones, biases, mask, wv, wo; then in-loop xt streams. ----
            wq_sb = fixed.tile([128, KT, COLS], BF16, name="wq", tag="wq")
            xt0 = xs.tile([128, KT, 512], BF16, name="xt", tag="xt")
            for q4 in range(4):
                ks = slice(q4 * 4, q4 * 4 + 4)
                rs_ = slice(q4 * 512, q4 * 512 + 512)
                nc.sync.dma_start(
                    out=wq_sb[:, ks, :],
                    in_=wq_d[rs_, :].rearrange("(kt p) c -> p kt c", p=128))
                nc.sync.dma_start(
                    out=xt0[:, ks, :],
                    in_=xT_d[0, rs_, 0:512].rearrange("(kt p) n -> p kt n", p=128))

            wk_sb = fixed.tile([128, KT, COLS], BF16, name="wk", tag="wk")
            nc.sync.dma_start(
                out=wk_sb, in_=wk_d[:, :].rearrange("(kt p) c -> p kt c", p=128))
            cos_sb = fixed.tile([64, L], F32, name="cos", tag="cos")
            nc.sync.dma_start(out=cos_sb, in_=cos_d[:, :])
            sin_sb = fixed.tile([64, L], F32, name="sin", tag="sin")
            nc.sync.dma_start(out=sin_sb, in_=sin_d[:, :])
            ones = fixed.tile([128, 512], BF16, name="ones", tag="ones")
            nc.sync.dma_start(out=ones, in_=ones_d[:, :])
            bq_sb = fixed.tile([1, COLS], BF16, name="bq", tag="bq")
            nc.sync.dma_start(out=bq_sb, in_=bq_d[:, :])
            bk_sb = fixed.tile([1, COLS], BF16, name="bk", tag="bk")
            nc.sync.dma_start(out=bk_sb, in_=bk_d[:, :])
            bv_sb = fixed.tile([1, COLS], BF16, name="bv", tag="bv")
            nc.sync.dma_start(out=bv_sb, in_=bv_d[:, :])
            mask_sb = fixed.tile([128, 128], BF16, name="mask", tag="mask")
            nc.sync.dma_start(out=mask_sb, in_=mask_d[:, :])
            id_sb = fixed.tile([128, 128], BF16, name="idm", tag="idm")
            nc.sync.dma_start(out=id_sb, in_=id_d[:, :])
            wv_sb = fixed.tile([128, KT, COLS], BF16, name="wv", tag="wv")
            nc.sync.dma_start(
                out=wv_sb, in_=wv_d[:, :].rearrange("(kt p) c -> p kt c", p=128))
            # wo's DMA is emitted after x(b0,c1)'s so it doesn't delay it
            wo_sb = fixed.tile([128, HPC, E], BF16, name="wo", tag="wo")

            qT = [qkvp.tile([128, L], BF16, name=f"qT{h}", tag=f"qT{h}") for h in range(HPC)]
            kT = [qkvp.tile([128, L], BF16, name=f"kT{h}", tag=f"kT{h}") for h in range(HPC)]
            oT = [qkvp.tile([128, L], BF16, name=f"oT{h}", tag=f"oT{h}") for h in range(HPC)]
            vv = qkvp.tile([128, 16, COLS], BF16, name="vv", tag="vv")  # [m-part, mb, cols]

            def attn_begin(lc, h):
                """Allocate av/rs + build the st/av emitters for one
                (l-chunk, head) attention stream."""
                nmb = 4 * lc + 4
                av = pav.tile([128, 512], F32, name="av", tag="av")
                rsp = prs.tile([128, 512], F32, name="rs", tag="rs")
                pts = [None] * nmb

                def emit_st(mb):
                    l0 = max(lc * 512, mb * 128)
                    npr = lc * 512 + 512 - l0
                    diag = mb >= 4 * lc
                    st = pA.tile([128, 512], F32, name="st", tag="A")
                    nc.tensor.matmul(
                        st[:, 0:npr],
                        lhsT=kT[h][:, mb * 128:(mb + 1) * 128],
                        rhs=qT[h][:, l0:l0 + npr], start=True,
                        stop=not diag)
                    if diag:
                        # causal mask folded in on PE: st += I @ mask
                        nc.tensor.matmul(
                            st[:, 0:128], lhsT=id_sb, rhs=mask_sb,
                            start=False, stop=True)
                    pt = ptp.tile([128, 512], BF16, name="pt", tag="pt")
                    nc.scalar.activation(
                        out=pt[:, 0:npr], in_=st[:, 0:npr], func=Exp)
                    pts[mb] = (pt, npr)

                def emit_av(mb):
                    pt, npr = pts[mb]
                    c0 = 512 - npr
                    nc.tensor.matmul(
                        av[:, c0:512],
                        lhsT=vv[:, mb, h * 128:(h + 1) * 128],
                        rhs=pt[:, 0:npr], start=(mb == 0),
                        stop=(mb == nmb - 1))
                    nc.tensor.matmul(
                        rsp[:, c0:512], lhsT=ones[:, 0:128],
                        rhs=pt[:, 0:npr], start=(mb == 0),
                        stop=(mb == nmb - 1))

                return av, rsp, emit_st, emit_av

            # pending out-proj tiles: list of (b, lc, eb, split_copy)
            op_queue = []

            def drain_op(n, alternate=False):
                # emits one eb-PAIR per unit of n: two yp tiles staged into a
                # shared ys2, written out with a single descriptor
                for j in range(n):
                    if not op_queue:
                        return
                    ys2 = ysp.tile([128, 2, 512], BF16, name="ys", tag="ys")
                    b_, lc_, eb0 = op_queue[0]
                    lsl = slice(lc_ * 512, (lc_ + 1) * 512)
                    for t in range(2):
                        b_, lc_, eb = op_queue.pop(0)
                        if alternate:
                            yp = (pA if t == 1 else pB).tile(
                                [128, 512], F32, name="yp",
                                tag="A" if t == 1 else "B")
                        else:
                            yp = pB.tile([128, 512], F32, name="yp", tag="B")
                        nc.tensor.matmul(
                            yp, lhsT=wo_sb[:, 0, eb * 128:(eb + 1) * 128],
                            rhs=oT[0][:, lsl], start=True, stop=False)
                        nc.tensor.matmul(
                            yp, lhsT=wo_sb[:, 1, eb * 128:(eb + 1) * 128],
                            rhs=oT[1][:, lsl], start=False, stop=True)
                        if alternate and t == 0:
                            nc.scalar.copy(out=ys2[:, t, :], in_=yp)
                        else:
                            nc.vector.tensor_copy(ys2[:, t, :], yp)
                    nc.sync.dma_start(
                        out=y_d[b_, eb0 * 128:(eb0 + 2) * 128, lsl]
                        .rearrange("(t p) l -> p t l", p=128),
                        in_=ys2)

            def rope(ps, dst, csl):
                # ps: [128, 512] f32 PSUM (head-dim major); writes bf16 dst
                t1 = ropep.tile([128, 512], F32, name="ropet1", tag="ropet1")
                nc.vector.scalar_tensor_tensor(
                    out=t1[0:64, :], in0=ps[64:128, :], scalar=-1.0,
                    in1=sin_sb[:, csl], op0=AluOpType.mult,
                    op1=AluOpType.mult)
                nc.vector.tensor_mul(
                    t1[64:128, :], ps[0:64, :], sin_sb[:, csl])
                t2 = ropep.tile([128, 512], F32, name="ropet2", tag="ropet2")
                nc.vector.tensor_mul(t2[0:64, :], ps[0:64, :], cos_sb[:, csl])
                nc.vector.tensor_mul(t2[64:128, :], ps[64:128, :], cos_sb[:, csl])
                nc.vector.tensor_add(dst, t1, t2)

            xt_pre = {}

            for b in range(B):
                # ---------- QKV projection: 512-token chunks, 3 passes ----
                for c in range(LC):
                    if b == 0 and c == 0:
                        xt = xt0
                    elif (b, c) in xt_pre:
                        xt = xt_pre.pop((b, c))
                    else:
                        xt = xs.tile([128, KT, 512], BF16, name="xt", tag="xt")
                        nc.sync.dma_start(
                            out=xt,
                            in_=xT_d[b, :, c * 512:(c + 1) * 512]
                            .rearrange("(kt p) n -> p kt n", p=128))
                        if b == 0 and c == 1:
                            nc.sync.dma_start(
                                out=wo_sb,
                                in_=wo_d[:, :].rearrange("(h p) e -> p h e", p=128))
                    csl = slice(c * 512, (c + 1) * 512)
                    # q pass
                    qps = [pA.tile([128, 512], F32, name=f"qps{h}", tag="A")
                           for h in range(HPC)]
                    for k in range(KT):
                        for h in range(HPC):
                            nc.tensor.matmul(
                                qps[h], lhsT=wq_sb[:, k, h * 128:(h + 1) * 128],
                                rhs=xt[:, k, :], start=(k == 0), stop=False)
                        if k in (5, 11):
                            drain_op(1)
                    for h in range(HPC):
                        nc.tensor.matmul(
                            qps[h], lhsT=bq_sb[0:1, h * 128:(h + 1) * 128],
                            rhs=ones[0:1, :], start=False, stop=True)
                    for h in range(HPC):
                        rope(qps[h], qT[h][:, csl], csl)
                    # k pass
                    kps = [pA.tile([128, 512], F32, name=f"kps{h}", tag="A")
                           for h in range(HPC)]
                    for k in range(KT):
                        for h in range(HPC):
                            nc.tensor.matmul(
                                kps[h], lhsT=wk_sb[:, k, h * 128:(h + 1) * 128],
                                rhs=xt[:, k, :], start=(k == 0), stop=False)
                        if k in (5, 11):
                            drain_op(1)
                    for h in range(HPC):
                        nc.tensor.matmul(
                            kps[h], lhsT=bk_sb[0:1, h * 128:(h + 1) * 128],
                            rhs=ones[0:1, :], start=False, stop=True)
                    for h in range(HPC):
                        rope(kps[h], kT[h][:, csl], csl)
                    # v pass: 4 x 128-token sub-tiles over the B banks
                    for i in range(4):
                        if c == LC - 1 and i == 3:
                            # prefill attention (lc0,h0)'s first 3 st/exp
                            # blocks so the st->exp->av chain is already hot
                            # when attention starts
                            attn_pre = attn_begin(0, 0)
                            for mb in range(3):
                                attn_pre[2](mb)
                        vps = pB.tile([128, COLS], F32, name="vps", tag="B")
                        t0 = i * 128
                        for k in range(KT):
                            nc.tensor.matmul(
                                vps, lhsT=xt[:, k, t0:t0 + 128],
                                rhs=wv_sb[:, k, :], start=(k == 0), stop=False)
                        nc.tensor.matmul(
                            vps, lhsT=ones[0:1, 0:128],
                            rhs=bv_sb[0:1, :], start=False, stop=True)
                        nc.scalar.copy(out=vv[:, c * 4 + i, :], in_=vps)

                # prefetch next batch's first x chunk during attention
                if b + 1 < B:
                    xt_n = xs.tile([128, KT, 512], BF16, name="xt", tag="xt")
                    nc.sync.dma_start(
                        out=xt_n,
                        in_=xT_d[b + 1, :, 0:512]
                        .rearrange("(kt p) n -> p kt n", p=128))
                    xt_pre[(b + 1, 0)] = xt_n

                # ---------- attention (S^T layout, causal); heads
                #            sequential, depth-3 pipeline; out-proj for
                #            lc-1 drains into attention gaps ----------
                for lc in range(LC):
                    nmb = 4 * lc + 4
                    lsl = slice(lc * 512, (lc + 1) * 512)
                    for h in range(HPC):
                        if lc == 0 and h == 0:
                            av, rsp, emit_st, emit_av = attn_pre
                            prefilled = 3
                        else:
                            av, rsp, emit_st, emit_av = attn_begin(lc, h)
                            prefilled = 0

                        depth = min(3, nmb)
                        for mb in range(prefilled, depth):
                            emit_st(mb)
                        for mb in range(nmb):
                            if mb + depth < nmb:
                                emit_st(mb + depth)
                            emit_av(mb)
                            if mb >= 3:
                                # keep 4 tiles back at the very end so they
                                # pad the last lc's normalization latency
                                hold = 4 if (b == B - 1 and lc == LC - 1) else 0
                                if len(op_queue) > hold:
                                    drain_op(1)

                        rec = smallp.tile([128, 512], F32, name="rec", tag="rec")
                        nc.vector.reciprocal(out=rec, in_=rsp)
                        nc.vector.tensor_mul(oT[h][:, lsl], av, rec)
                        hold = (4 if (b == B - 1 and lc == LC - 1 and h == 0)
                                else 0)
                        drain_op(min(3, max(0, (len(op_queue) - hold) // 2)))
                    # queue out-proj for this lc once both heads normalized
                    op_queue.extend((b, lc, eb) for eb in range(KT))
                # drain everything still pending for the last batch
                if b == B - 1:
                    drain_op((len(op_queue) + 1) // 2, alternate=True)
    nc.compile()
    return nc


_NC_CACHE = None


def kernel(x, Wq, bq, Wk, bk, Wv, bv, Wo, bo):
    global _NC_CACHE
    import ml_dtypes
    from concourse.bass_utils import run_bass_kernel_spmd

    BF = ml_dtypes.bfloat16
    x = np.asarray(x, np.float32)
    scale = HD ** (-0.5)

    inv = 1.0 / (BASE ** (np.arange(0, HD, 2, dtype=np.float32) / HD))
    fr = np.outer(inv, np.arange(L, dtype=np.float32))  # [64, L]
    cosf = np.cos(fr).astype(np.float32)
    sinf = np.sin(fr).astype(np.float32)
    mask = np.where(np.arange(128)[:, None] <= np.arange(128)[None, :],
                    0.0, NEG).astype(BF)
    idm = np.eye(128, dtype=np.float32).astype(BF)

    xT = np.ascontiguousarray(np.transpose(x, (0, 2, 1))).astype(BF)  # [B, E, L]

    in_maps = []
    for c in range(NCORES):
        cols = slice(c * COLS, (c + 1) * COLS)
        bq_c = (np.asarray(bq, np.float32)[cols] * scale).astype(BF)[None, :]
        bk_c = np.asarray(bk, np.float32)[cols].astype(BF)[None, :]
        bv_c = np.asarray(bv, np.float32)[cols].astype(BF)[None, :]
        in_maps.append({
            "xT": xT,
            "wq": (np.ascontiguousarray(np.asarray(Wq, np.float32)[:, cols])
                   * scale).astype(BF),
            "wk": np.ascontiguousarray(np.asarray(Wk, np.float32)[:, cols]).astype(BF),
            "wv": np.ascontiguousarray(np.asarray(Wv, np.float32)[:, cols]).astype(BF),
            "wo": np.ascontiguousarray(np.asarray(Wo, np.float32)[cols, :]).astype(BF),
            "bq": bq_c, "bk": bk_c, "bv": bv_c,
            "cosf": cosf,
            "sinf": sinf,
            "mask": mask,
            "idm": idm,
            "ones": np.ones((128, 512), BF),
        })

    if _NC_CACHE is None:
        _NC_CACHE = _build_program()
    res = run_bass_kernel_spmd(_NC_CACHE, in_maps, list(range(NCORES)))
    acc = np.zeros((B, E, L), np.float64)
    for c in range(NCORES):
        acc += np.asarray(res.results[c]["yT"], dtype=np.float32)
    y = np.transpose(acc, (0, 2, 1)).astype(np.float32) + np.asarray(bo, np.float32)
    return y
